# revision 47
# baseline (speedup 1.0000x reference)
"""Dense multi-head attention (S=4096, H=16, D=64) on 8 Trainium2 NeuronCores.

Sharding: heads split across cores (2 heads per core), no cross-core comms.

Host side: Q and K are pre-transposed per head to [D, S] (d-major) so the
kernel DMA-loads Q^T/K^T directly with 16KB-contiguous runs; V stays [S, D].
Q and K are additionally pre-scaled by G = sqrt(1024*log2(e)/sqrt(D)) so the
PSUM logits arrive as x = 1024*log2(e)*s' (s' = scaled logit), which both
exp paths consume directly.

Per-core kernel (per head):
  - QK runs in 64x128 row-tiled PE mode: even k-tiles' K^T stationaries live
    on SBUF/array rows 0-63, odd k-tiles' on rows 64-127, and Q^T is
    duplicated into both partition halves. The two 512-wide matmuls of a
    k-tile pair execute concurrently on the two array halves (contraction is
    only D=64), halving QK's PE time vs the padded-to-128 formulation.
  - exp of each pair's [128, 1024] PSUM group alternates between ScalarE
    (table exp, scale/bias fused) and a custom 8-block VectorE op
    (EXP2_FAST_ANT: magic-number round + quadratic exp2 mantissa polynomial,
    result written as int16 bit pattern = fp16 value). Both paths produce
    kappa*exp(s'): the common factor cancels in the softmax divide.
    Splitting exp across two engines removes the ScalarE bottleneck
    (baseline: ACT 256us busy of 304us total).
  - PV also runs row-tiled 64x128: each k-tile's 128-key contraction splits
    into keys 0-63 (array rows 0-63, accumulating into acc_lo) and keys
    64-127 (rows 64-127, acc_hi); the two matmuls execute concurrently, so
    the pair costs one 512-column stream. Keeping QK and PV in the same PE
    tiling mode avoids the ~250ns/pair LDWEIGHTS exposure that mode
    alternation costs (measured v3). V' has an appended ones-column so row
    64 of acc_lo+acc_hi is the softmax denominator. PV lags exp by two
    groups so the in-order PE stream never waits on an exp in flight.
  - Epilogue: acc_lo+acc_hi -> fp16 (ScalarE copy + VectorE add, one PSUM
    read each), XBAR DMA-transpose to [128 q, 4, 80], reciprocal of col 64,
    broadcast multiply, DMA out. Epilogue DMAs issue via the ScalarE HWDGE
    path so they never queue behind stalled input loads on the SP sequencer.

Pipeline notes (all measured on HW via neuron-profile traces):
  - The in-order PE queue forces PV to lag exp by two pair-groups, else PE
    idles on every exp's ~1.2us latency.
  - Input loads are ordered by consumption deadline (K chunks + V first),
    with generous stg buffering -- strict-FIFO sequencers turn any stalled
    load into head-of-line blocking of the whole pipeline.
"""

import numpy as np

import concourse.mybir as mybir
import concourse.tile as tile
from concourse import bacc
from concourse.bass_utils import run_bass_kernel_spmd

S = 4096
H = 16
D = 64
NCORES = 8
HPC = H // NCORES  # heads per core
NKT = S // 128  # 32 k-tiles per head
NPAIR = NKT // 2  # 16 row-tiled k-tile pairs per head
NQC = S // 512  # 8 q chunks per head
NCH = NKT // 8  # 4 load chunks per head (1024 columns each)
SCALE = 1.0 / np.sqrt(D)

F32 = mybir.dt.float32
F16 = mybir.dt.float16
I16 = mybir.dt.int16

# ---- exp formulation constants -------------------------------------------
LOG2E = float(np.log2(np.e))
PRESCALE = float(np.sqrt(1024.0 * LOG2E * SCALE))  # host-applied to q and k
MAGIC = 12884901888.0  # 1.5 * 2^33: fp32 add rounds x to a multiple of 1024
EXP_A = 0.98531599  # quadratic exp2 mantissa poly (minimax incl. borrow)
EXP_B = -3.0719600e-4
MEAN_ADJ = 0.00577  # DVE op's mean rel error, matched in the ACT path
SCALE_ACT = float(np.log(2.0) / 1024.0)
# Both exp paths produce kappa*exp(s') with kappa = 1.01587 (the DVE fit's
# common scale); it cancels in the softmax divide.
BIAS_ACT = float(np.log(0.71833097 * np.sqrt(2.0)) + MEAN_ADJ)

# ---- custom DVE op: EXP2_FAST_ANT ----------------------------------------
# out_i16 = tb + P where (all fp32, one 8-block DVE pass, 1 elem/cycle/lane):
#   w  = x + MAGIC          # rounds x to nearest multiple of 1024
#   u  = w - MAGIC          # 1024*k
#   f  = x - u              # mantissa residual in [-512, 512)
#   tb = w - (MAGIC-15360)  # 1024*(15+k): fp16 exponent field
#   P  = f*(EXP_A + f*EXP_B)
# int16(out) bitcast fp16 == kappa * exp(s') to ~1.6% (sawtooth), which the
# softmax normalization reduces to ~6e-3 of output max.


def _exp_reference(in0, in1, s0, s1, imm2):
    x = in0.astype(np.float32)
    w = (x + np.float32(s0)).astype(np.float32)
    u = (w - np.float32(s0)).astype(np.float32)
    f = (x - u).astype(np.float32)
    tb = (w - np.float32(s1)).astype(np.float32)
    return (tb + f * (np.float32(imm2) + f * in1)).astype(np.float32)


def _register_exp_op():
    import concourse.dve_ops as dops
    from concourse.dve_spec import (
        C0,
        C1,
        C2,
        C3,
        Spec,
        Src0,
        _has_src1,
        _spill_c3_to_src1,
        lower,
    )
    from concourse.dve_uop import DveOpSpec

    name = "EXP2_FAST_ANT"
    for op in dops.OPS:
        if op.name == name:
            return op
    w = Src0 + C0
    u = w - C0
    f = Src0 - u
    tb = w - C1
    P = (f * C3 + C2) * f
    body = _spill_c3_to_src1(tb + P)
    spec = Spec(body=body, reference=_exp_reference)
    row = max(dops._SUB_OPCODE_FOR_NAME.values()) + 1
    assert row < 0x20
    dops._SUB_OPCODE_FOR_NAME[name] = row
    shas = {}
    for ver in ("v3", "v4"):
        tmp = DveOpSpec(
            name=name, opcode=row, uops=lower(spec, ver=ver), rd1_en=_has_src1(spec)
        )
        shas[ver] = tmp.sha(ver)
    op = dops.DveOp(name, spec, subdim=False, uops_sha=shas)
    dops.OPS.append(op)
    dops.CUSTOM_DVE_SPECS[name] = spec
    return op


EXP_OP = _register_exp_op()


EXP_PAT = "AD"  # pure alternation: seamless, chunk-aligned 50/50 split


def _build_head(nc, tc, pools, bco, bact, q, k, v, o, h):
    sb, epool, spsum, opsum = pools

    # ---- Phase A: direct fp16 DMAs; host delivers device-ready layouts ----
    # kt: even k-tiles' K^T on partitions 0..63, odd k-tiles' on 64..127
    # (host-interleaved). qt: Q^T duplicated into both partition halves
    # (host-duplicated). vstage: V' with ones column 64 and zero padding
    # (host-padded). Loads ordered by consumption deadline: every kts chunk
    # is read within q-chunk 0, V' feeds PV from slot 2, qts[b] waits until
    # q-chunk 2b.
    qts = [
        sb.tile([128, 1024], F16, tag=f"qt{b}", name=f"qt{b}") for b in range(NCH)
    ]
    kts = [
        sb.tile([128, 1024], F16, tag=f"kt{b}", name=f"kt{b}") for b in range(NCH)
    ]

    def qk_chunk(src_t, dsts, b, c0=0, w=1024):
        nc.sync.dma_start(
            dsts[b][:, c0 : c0 + w],
            src_t.ap()[h, :, b * 1024 + c0 : b * 1024 + c0 + w],
        )

    qk_chunk(k, kts, 0, c0=0, w=512)
    qk_chunk(q, qts, 0, c0=0, w=512)

    vstage = sb.tile([128, NKT, 128], F16, tag="vstage")
    nc.sync.dma_start(vstage[:], v.ap()[h])

    qk_chunk(k, kts, 0, c0=512, w=512)
    qk_chunk(k, kts, 1)
    qk_chunk(k, kts, 2)
    qk_chunk(k, kts, 3)
    qk_chunk(q, qts, 0, c0=512, w=512)
    for b in range(1, NCH):
        qk_chunk(q, qts, b)

    # ---- Phase B: attention, PV lagging exp by two pair-groups ----
    def qk_group(qc, p):
        t0 = 2 * p
        b = t0 // 8
        qb = qc // 2
        qcols = (qc * 512) % 1024
        sp = spsum.tile([128, 1024], F32, tag="sp")
        nc.tensor.matmul(
            sp[:, 0:512],
            kts[b][0:D, (t0 % 8) * 128 : (t0 % 8 + 1) * 128],
            qts[qb][0:D, qcols : qcols + 512],
        )
        nc.tensor.matmul(
            sp[:, 512:1024],
            kts[b][64 : 64 + D, (t0 % 8 + 1) * 128 : (t0 % 8 + 2) * 128],
            qts[qb][64 : 64 + D, qcols : qcols + 512],
        )
        return sp

    def epilogue(ot, qs):
        # XBAR DMA transpose: ot [80, 512] fp16 -> otT [128, 4, 80] where
        # query q = n*128 + p sits at otT[p, n, :] (rows 65..79 are junk pad).
        otT = sb.tile([128, 4, 80], F16, tag="otT", bufs=3)
        nc.scalar.dma_start_transpose(otT[:], ot[:])
        fin = sb.tile([128, 4, D], F32, tag="fin", bufs=3)
        rcp = sb.tile([128, 4, 1], F32, tag="rcp")
        nc.vector.reciprocal(rcp[:, :, 0], otT[:, :, D])
        nc.vector.tensor_tensor(
            fin[:],
            otT[:, :, 0:D],
            rcp[:].to_broadcast([128, 4, D]),
            mybir.AluOpType.mult,
        )
        nc.scalar.dma_start(
            o.ap()[h, qs : qs + 512, :].rearrange("(n p) d -> p n d", p=128),
            fin[:],
        )

    groups = [(qc, p) for qc in range(NQC) for p in range(NPAIR)]
    n = len(groups)
    ets = {}
    state = {"acc": None, "pending": None}

    def issue_exp(i, sp):
        qc, p = groups[i]
        et = epool.tile([128, 1024], F16, tag="et")
        if EXP_PAT[i % len(EXP_PAT)] == "D":
            # VectorE path: custom 8-block exp, int16 bit pattern = fp16
            nc.vector._custom_dve(
                EXP_OP,
                out=et[:].bitcast(I16),
                in0=sp[:],
                in1=bco[:, 0:1],
                s0=MAGIC,
                s1=MAGIC - 15360.0,
                imm2=EXP_A,
            )
        else:
            nc.scalar.activation(
                et[:],
                sp[:],
                mybir.ActivationFunctionType.Exp,
                scale=SCALE_ACT,
                bias=bact[:, 0:1],
            )
        ets[i] = et

    def issue_pv(i):
        qc, p = groups[i]
        et = ets.pop(i)
        if p == 0:
            if state["pending"] is not None:
                epilogue(*state["pending"])
                state["pending"] = None
            state["acc"] = (
                opsum.tile([128, 512], F32, tag="acc", name="acc_lo"),
                opsum.tile([128, 512], F32, tag="acc", name="acc_hi"),
            )
        acc_lo, acc_hi = state["acc"]
        for j in range(2):
            t = 2 * p + j
            # Row-tiled PV pair: keys 0-63 on array rows 0-63 -> acc_lo,
            # keys 64-127 on rows 64-127 -> acc_hi; runs concurrently.
            nc.tensor.matmul(
                acc_lo[:],
                vstage[0:64, t, :],
                et[0:64, j * 512 : (j + 1) * 512],
                start=(t == 0),
                stop=(t == NKT - 1),
            )
            nc.tensor.matmul(
                acc_hi[:],
                vstage[64:128, t, :],
                et[64:128, j * 512 : (j + 1) * 512],
                start=(t == 0),
                stop=(t == NKT - 1),
            )
        if p == NPAIR - 1:
            # merge the two half-contraction accumulators; frees both banks
            # (an instruction may read only ONE input from PSUM: ScalarE
            # copies acc_lo out, VectorE adds acc_hi)
            ot = sb.tile([80, 512], F16, tag="ot", bufs=3)
            nc.scalar.copy(ot[0 : D + 1, :], acc_lo[0 : D + 1, :])
            nc.vector.tensor_tensor(
                ot[0 : D + 1, :],
                ot[0 : D + 1, :],
                acc_hi[0 : D + 1, :],
                mybir.AluOpType.add,
            )
            state["pending"] = (ot, qc * 512)

    # PV lags exp by two groups so the in-order PE stream (QK(i+1), PV(i-2))
    # never waits on an exp still in flight on ScalarE/VectorE.
    sp_next = qk_group(*groups[0])
    pvq = []
    for i in range(n):
        issue_exp(i, sp_next)
        if i + 1 < n:
            sp_next = qk_group(*groups[i + 1])
        if i >= 2:
            pvq.append(i - 2)
            # Hold a chunk-first PV one extra slot: one more QK pair runs
            # before the PE needs the acc banks the merge chain is freeing.
            while pvq and not (
                len(pvq) == 1 and groups[pvq[0]][1] == 0 and i + 1 < n
            ):
                issue_pv(pvq.pop(0))
    for j in pvq + [n - 2, n - 1]:
        issue_pv(j)
    epilogue(*state["pending"])


def _build():
    nc = bacc.Bacc(trn_type="TRN2", debug=False, num_devices=NCORES)
    q = nc.dram_tensor("q", [HPC, 128, S], F16, kind="ExternalInput")
    k = nc.dram_tensor("k", [HPC, 128, S], F16, kind="ExternalInput")
    v = nc.dram_tensor("v", [HPC, 128, NKT, 128], F16, kind="ExternalInput")
    o = nc.dram_tensor("o", [HPC, S, D], F32, kind="ExternalOutput")

    with tile.TileContext(nc) as tc:
        with (
            tc.tile_pool(name="const", bufs=1) as cpool,
            tc.tile_pool(name="sb", bufs=2) as sb,
            tc.tile_pool(name="epool", bufs=6) as epool,
            tc.tile_pool(name="spsum", bufs=3, space="PSUM") as spsum,
            tc.tile_pool(name="opsum", bufs=2, space="PSUM") as opsum,
        ):
            # Dummy exp at t~0 pulls the ACT table-load DMA in front of the
            # input DMAs (otherwise the first input chunk queues behind it).
            warm = cpool.tile([128, 1], F32, tag="warm")
            nc.gpsimd.memset(warm[:], 0.0)
            nc.scalar.activation(
                warm[:], warm[:], mybir.ActivationFunctionType.Exp
            )
            bco = cpool.tile([128, 1], F32, tag="bco")
            nc.gpsimd.memset(bco[:], EXP_B)
            bact = cpool.tile([128, 1], F32, tag="bact")
            nc.gpsimd.memset(bact[:], BIAS_ACT)
            pools = (sb, epool, spsum, opsum)
            for h in range(HPC):
                _build_head(nc, tc, pools, bco, bact, q, k, v, o, h)

    nc.compile()
    return nc


_NC_CACHE = None


def prepare_in_maps(query, key, value):
    """Host prep: per-core slices, prescale q/k, cast fp16, and lay out in
    the exact SBUF formats (K even/odd row-interleave, Q duplicated halves,
    V' ones-column padded) so the kernel loads with plain DMAs."""
    query = np.asarray(query)
    key = np.asarray(key)
    value = np.asarray(value)
    g = np.float32(PRESCALE)
    in_maps = []
    for c in range(NCORES):
        sl = slice(c * HPC, (c + 1) * HPC)
        qh = ((query[:, sl, :] * g).transpose(1, 2, 0)).astype(np.float16)
        kh = ((key[:, sl, :] * g).transpose(1, 2, 0)).astype(np.float16)
        qhost = np.ascontiguousarray(np.concatenate([qh, qh], axis=1))
        kr = kh.reshape(HPC, D, NKT, 128)
        khost = np.zeros((HPC, 128, NKT, 128), np.float16)
        khost[:, 0:D, 0::2] = kr[:, :, 0::2]
        khost[:, 64 : 64 + D, 1::2] = kr[:, :, 1::2]
        vh = (
            value[:, sl, :]
            .transpose(1, 0, 2)
            .reshape(HPC, NKT, 128, D)
            .astype(np.float16)
        )
        vhost = np.zeros((HPC, 128, NKT, 128), np.float16)
        vhost[:, :, :, 0:D] = vh.transpose(0, 2, 1, 3)
        vhost[:, :, :, D] = np.float16(1.0)
        in_maps.append(
            {
                "q": qhost,
                "k": khost.reshape(HPC, 128, S),
                "v": vhost,
            }
        )
    return in_maps


def kernel(query, key, value):
    global _NC_CACHE
    if _NC_CACHE is None:
        _NC_CACHE = _build()
    nc = _NC_CACHE

    in_maps = prepare_in_maps(query, key, value)
    res = run_bass_kernel_spmd(nc, in_maps, core_ids=list(range(NCORES)))
    out = np.concatenate(
        [res.results[c]["o"].transpose(1, 0, 2) for c in range(NCORES)], axis=1
    )
    return out


# revision 49
# speedup vs baseline: 1.0125x; 1.0125x over previous
"""Dense multi-head attention (S=4096, H=16, D=64) on 8 Trainium2 NeuronCores.

Sharding: heads split across cores (2 heads per core), no cross-core comms.

Host side: Q and K are pre-transposed per head to [D, S] (d-major) so the
kernel DMA-loads Q^T/K^T directly with 16KB-contiguous runs; V stays [S, D].
Q and K are additionally pre-scaled by G = sqrt(1024*log2(e)/sqrt(D)) so the
PSUM logits arrive as x = 1024*log2(e)*s' (s' = scaled logit), which both
exp paths consume directly.

Per-core kernel (per head):
  - QK runs in 64x128 row-tiled PE mode: even k-tiles' K^T stationaries live
    on SBUF/array rows 0-63, odd k-tiles' on rows 64-127, and Q^T is
    duplicated into both partition halves. The two 512-wide matmuls of a
    k-tile pair execute concurrently on the two array halves (contraction is
    only D=64), halving QK's PE time vs the padded-to-128 formulation.
  - exp of each pair's [128, 1024] PSUM group alternates between ScalarE
    (table exp, scale/bias fused) and a custom 8-block VectorE op
    (EXP2_FAST_ANT: magic-number round + quadratic exp2 mantissa polynomial,
    result written as int16 bit pattern = fp16 value). Both paths produce
    kappa*exp(s'): the common factor cancels in the softmax divide.
    Splitting exp across two engines removes the ScalarE bottleneck
    (baseline: ACT 256us busy of 304us total).
  - PV also runs row-tiled 64x128: each k-tile's 128-key contraction splits
    into keys 0-63 (array rows 0-63, accumulating into acc_lo) and keys
    64-127 (rows 64-127, acc_hi); the two matmuls execute concurrently, so
    the pair costs one 512-column stream. Keeping QK and PV in the same PE
    tiling mode avoids the ~250ns/pair LDWEIGHTS exposure that mode
    alternation costs (measured v3). V' has an appended ones-column so row
    64 of acc_lo+acc_hi is the softmax denominator. PV lags exp by two
    groups so the in-order PE stream never waits on an exp in flight.
  - Epilogue: acc_lo+acc_hi -> fp16 (ScalarE copy + VectorE add, one PSUM
    read each), XBAR DMA-transpose to [128 q, 4, 80], reciprocal of col 64,
    broadcast multiply, DMA out. Epilogue DMAs issue via the ScalarE HWDGE
    path so they never queue behind stalled input loads on the SP sequencer.

Pipeline notes (all measured on HW via neuron-profile traces):
  - The in-order PE queue forces PV to lag exp by two pair-groups, else PE
    idles on every exp's ~1.2us latency.
  - Input loads are ordered by consumption deadline (K chunks + V first),
    with generous stg buffering -- strict-FIFO sequencers turn any stalled
    load into head-of-line blocking of the whole pipeline.
"""

import numpy as np

import concourse.mybir as mybir
import concourse.tile as tile
from concourse import bacc
from concourse.bass_utils import run_bass_kernel_spmd

S = 4096
H = 16
D = 64
NCORES = 8
HPC = H // NCORES  # heads per core
NKT = S // 128  # 32 k-tiles per head
NPAIR = NKT // 2  # 16 row-tiled k-tile pairs per head
NQC = S // 512  # 8 q chunks per head
NCH = NKT // 8  # 4 load chunks per head (1024 columns each)
SCALE = 1.0 / np.sqrt(D)

F32 = mybir.dt.float32
F16 = mybir.dt.float16
I16 = mybir.dt.int16

# ---- exp formulation constants -------------------------------------------
LOG2E = float(np.log2(np.e))
PRESCALE = float(np.sqrt(1024.0 * LOG2E * SCALE))  # host-applied to q and k
MAGIC = 12884901888.0  # 1.5 * 2^33: fp32 add rounds x to a multiple of 1024
EXP_A = 0.98531599  # quadratic exp2 mantissa poly (minimax incl. borrow)
EXP_B = -3.0719600e-4
MEAN_ADJ = 0.00577  # DVE op's mean rel error, matched in the ACT path
SCALE_ACT = float(np.log(2.0) / 1024.0)
# Both exp paths produce kappa*exp(s') with kappa = 1.01587 (the DVE fit's
# common scale); it cancels in the softmax divide.
BIAS_ACT = float(np.log(0.71833097 * np.sqrt(2.0)) + MEAN_ADJ)

# ---- custom DVE op: EXP2_FAST_ANT ----------------------------------------
# out_i16 = tb + P where (all fp32, one 8-block DVE pass, 1 elem/cycle/lane):
#   w  = x + MAGIC          # rounds x to nearest multiple of 1024
#   u  = w - MAGIC          # 1024*k
#   f  = x - u              # mantissa residual in [-512, 512)
#   tb = w - (MAGIC-15360)  # 1024*(15+k): fp16 exponent field
#   P  = f*(EXP_A + f*EXP_B)
# int16(out) bitcast fp16 == kappa * exp(s') to ~1.6% (sawtooth), which the
# softmax normalization reduces to ~6e-3 of output max.


def _exp_reference(in0, in1, s0, s1, imm2):
    x = in0.astype(np.float32)
    w = (x + np.float32(s0)).astype(np.float32)
    u = (w - np.float32(s0)).astype(np.float32)
    f = (x - u).astype(np.float32)
    tb = (w - np.float32(s1)).astype(np.float32)
    return (tb + f * (np.float32(imm2) + f * in1)).astype(np.float32)


def _register_exp_op():
    import concourse.dve_ops as dops
    from concourse.dve_spec import (
        C0,
        C1,
        C2,
        C3,
        Spec,
        Src0,
        _has_src1,
        _spill_c3_to_src1,
        lower,
    )
    from concourse.dve_uop import DveOpSpec

    name = "EXP2_FAST_ANT"
    for op in dops.OPS:
        if op.name == name:
            return op
    w = Src0 + C0
    u = w - C0
    f = Src0 - u
    tb = w - C1
    P = (f * C3 + C2) * f
    body = _spill_c3_to_src1(tb + P)
    spec = Spec(body=body, reference=_exp_reference)
    row = max(dops._SUB_OPCODE_FOR_NAME.values()) + 1
    assert row < 0x20
    dops._SUB_OPCODE_FOR_NAME[name] = row
    shas = {}
    for ver in ("v3", "v4"):
        tmp = DveOpSpec(
            name=name, opcode=row, uops=lower(spec, ver=ver), rd1_en=_has_src1(spec)
        )
        shas[ver] = tmp.sha(ver)
    op = dops.DveOp(name, spec, subdim=False, uops_sha=shas)
    dops.OPS.append(op)
    dops.CUSTOM_DVE_SPECS[name] = spec
    return op


EXP_OP = _register_exp_op()


EXP_PAT = "ADADADADADADADA"  # 8 ScalarE / 7 VectorE per 15 exp groups


def _build_head(nc, tc, pools, bco, bact, q, k, v, o, h):
    sb, epool, spsum, opsum = pools

    # ---- Phase A: direct fp16 DMAs; host delivers device-ready layouts ----
    # kt: even k-tiles' K^T on partitions 0..63, odd k-tiles' on 64..127
    # (host-interleaved). qt: Q^T duplicated into both partition halves
    # (host-duplicated). vstage: V' with ones column 64 and zero padding
    # (host-padded). Loads ordered by consumption deadline: every kts chunk
    # is read within q-chunk 0, V' feeds PV from slot 2, qts[b] waits until
    # q-chunk 2b.
    qts = [
        sb.tile([128, 1024], F16, tag=f"qt{b}", name=f"qt{b}") for b in range(NCH)
    ]
    kts = [
        sb.tile([128, 1024], F16, tag=f"kt{b}", name=f"kt{b}") for b in range(NCH)
    ]

    def qk_chunk(src_t, dsts, b, c0=0, w=1024):
        nc.sync.dma_start(
            dsts[b][:, c0 : c0 + w],
            src_t.ap()[h, :, b * 1024 + c0 : b * 1024 + c0 + w],
        )

    qk_chunk(k, kts, 0, c0=0, w=512)
    qk_chunk(q, qts, 0, c0=0, w=512)

    vstage = sb.tile([128, NKT, 128], F16, tag="vstage")
    nc.sync.dma_start(vstage[:], v.ap()[h])

    qk_chunk(k, kts, 0, c0=512, w=512)
    qk_chunk(k, kts, 1)
    qk_chunk(k, kts, 2)
    qk_chunk(k, kts, 3)
    qk_chunk(q, qts, 0, c0=512, w=512)
    for b in range(1, NCH):
        qk_chunk(q, qts, b)

    # ---- Phase B: attention, PV lagging exp by two pair-groups ----
    def qk_group(qc, p):
        t0 = 2 * p
        b = t0 // 8
        qb = qc // 2
        qcols = (qc * 512) % 1024
        sp = spsum.tile([128, 1024], F32, tag="sp")
        nc.tensor.matmul(
            sp[:, 0:512],
            kts[b][0:D, (t0 % 8) * 128 : (t0 % 8 + 1) * 128],
            qts[qb][0:D, qcols : qcols + 512],
        )
        nc.tensor.matmul(
            sp[:, 512:1024],
            kts[b][64 : 64 + D, (t0 % 8 + 1) * 128 : (t0 % 8 + 2) * 128],
            qts[qb][64 : 64 + D, qcols : qcols + 512],
        )
        return sp

    def epilogue(ot, qs):
        # XBAR DMA transpose: ot [80, 512] fp16 -> otT [128, 4, 80] where
        # query q = n*128 + p sits at otT[p, n, :] (rows 65..79 are junk pad).
        otT = sb.tile([128, 4, 80], F16, tag="otT", bufs=3)
        nc.sync.dma_start_transpose(otT[:], ot[:])
        fin = sb.tile([128, 4, D], F32, tag="fin", bufs=3)
        rcp = sb.tile([128, 4, 1], F32, tag="rcp")
        nc.vector.reciprocal(rcp[:, :, 0], otT[:, :, D])
        nc.vector.tensor_tensor(
            fin[:],
            otT[:, :, 0:D],
            rcp[:].to_broadcast([128, 4, D]),
            mybir.AluOpType.mult,
        )
        nc.sync.dma_start(
            o.ap()[h, qs : qs + 512, :].rearrange("(n p) d -> p n d", p=128),
            fin[:],
        )

    groups = [(qc, p) for qc in range(NQC) for p in range(NPAIR)]
    n = len(groups)
    ets = {}
    state = {"acc": None, "pending": None}

    def issue_exp(i, sp):
        qc, p = groups[i]
        et = epool.tile([128, 1024], F16, tag="et")
        if EXP_PAT[i % len(EXP_PAT)] == "D":
            # VectorE path: custom 8-block exp, int16 bit pattern = fp16
            nc.vector._custom_dve(
                EXP_OP,
                out=et[:].bitcast(I16),
                in0=sp[:],
                in1=bco[:, 0:1],
                s0=MAGIC,
                s1=MAGIC - 15360.0,
                imm2=EXP_A,
            )
        else:
            nc.scalar.activation(
                et[:],
                sp[:],
                mybir.ActivationFunctionType.Exp,
                scale=SCALE_ACT,
                bias=bact[:, 0:1],
            )
        ets[i] = et

    def issue_pv(i):
        qc, p = groups[i]
        et = ets.pop(i)
        if p == 0:
            if state["pending"] is not None:
                epilogue(*state["pending"])
                state["pending"] = None
            state["acc"] = (
                opsum.tile([128, 512], F32, tag="acc", name="acc_lo"),
                opsum.tile([128, 512], F32, tag="acc", name="acc_hi"),
            )
        acc_lo, acc_hi = state["acc"]
        for j in range(2):
            t = 2 * p + j
            # Row-tiled PV pair: keys 0-63 on array rows 0-63 -> acc_lo,
            # keys 64-127 on rows 64-127 -> acc_hi; runs concurrently.
            nc.tensor.matmul(
                acc_lo[:],
                vstage[0:64, t, :],
                et[0:64, j * 512 : (j + 1) * 512],
                start=(t == 0),
                stop=(t == NKT - 1),
            )
            nc.tensor.matmul(
                acc_hi[:],
                vstage[64:128, t, :],
                et[64:128, j * 512 : (j + 1) * 512],
                start=(t == 0),
                stop=(t == NKT - 1),
            )
        if p == NPAIR - 1:
            # merge the two half-contraction accumulators; frees both banks
            # (an instruction may read only ONE input from PSUM: ScalarE
            # copies acc_lo out, VectorE adds acc_hi)
            ot = sb.tile([80, 512], F16, tag="ot", bufs=3)
            nc.scalar.copy(ot[0 : D + 1, :], acc_lo[0 : D + 1, :])
            nc.vector.tensor_tensor(
                ot[0 : D + 1, :],
                ot[0 : D + 1, :],
                acc_hi[0 : D + 1, :],
                mybir.AluOpType.add,
            )
            state["pending"] = (ot, qc * 512)

    # PV lags exp by two groups so the in-order PE stream (QK(i+1), PV(i-2))
    # never waits on an exp still in flight on ScalarE/VectorE.
    sp_next = qk_group(*groups[0])
    pvq = []
    for i in range(n):
        issue_exp(i, sp_next)
        if i + 1 < n:
            sp_next = qk_group(*groups[i + 1])
        if i >= 2:
            pvq.append(i - 2)
            # Hold a chunk-first PV one extra slot: one more QK pair runs
            # before the PE needs the acc banks the merge chain is freeing.
            while pvq and not (
                len(pvq) == 1 and groups[pvq[0]][1] == 0 and i + 1 < n
            ):
                issue_pv(pvq.pop(0))
    for j in pvq + [n - 2, n - 1]:
        issue_pv(j)
    epilogue(*state["pending"])


def _build():
    nc = bacc.Bacc(trn_type="TRN2", debug=False, num_devices=NCORES)
    q = nc.dram_tensor("q", [HPC, 128, S], F16, kind="ExternalInput")
    k = nc.dram_tensor("k", [HPC, 128, S], F16, kind="ExternalInput")
    v = nc.dram_tensor("v", [HPC, 128, NKT, 128], F16, kind="ExternalInput")
    o = nc.dram_tensor("o", [HPC, S, D], F32, kind="ExternalOutput")

    with tile.TileContext(nc) as tc:
        with (
            tc.tile_pool(name="const", bufs=1) as cpool,
            tc.tile_pool(name="sb", bufs=2) as sb,
            tc.tile_pool(name="epool", bufs=6) as epool,
            tc.tile_pool(name="spsum", bufs=3, space="PSUM") as spsum,
            tc.tile_pool(name="opsum", bufs=2, space="PSUM") as opsum,
        ):
            # Dummy exp at t~0 pulls the ACT table-load DMA in front of the
            # input DMAs (otherwise the first input chunk queues behind it).
            warm = cpool.tile([128, 1], F32, tag="warm")
            nc.gpsimd.memset(warm[:], 0.0)
            nc.scalar.activation(
                warm[:], warm[:], mybir.ActivationFunctionType.Exp
            )
            bco = cpool.tile([128, 1], F32, tag="bco")
            nc.gpsimd.memset(bco[:], EXP_B)
            bact = cpool.tile([128, 1], F32, tag="bact")
            nc.gpsimd.memset(bact[:], BIAS_ACT)
            pools = (sb, epool, spsum, opsum)
            for h in range(HPC):
                _build_head(nc, tc, pools, bco, bact, q, k, v, o, h)

    nc.compile()
    return nc


_NC_CACHE = None


def prepare_in_maps(query, key, value):
    """Host prep: per-core slices, prescale q/k, cast fp16, and lay out in
    the exact SBUF formats (K even/odd row-interleave, Q duplicated halves,
    V' ones-column padded) so the kernel loads with plain DMAs."""
    query = np.asarray(query)
    key = np.asarray(key)
    value = np.asarray(value)
    g = np.float32(PRESCALE)
    in_maps = []
    for c in range(NCORES):
        sl = slice(c * HPC, (c + 1) * HPC)
        qh = ((query[:, sl, :] * g).transpose(1, 2, 0)).astype(np.float16)
        kh = ((key[:, sl, :] * g).transpose(1, 2, 0)).astype(np.float16)
        qhost = np.ascontiguousarray(np.concatenate([qh, qh], axis=1))
        kr = kh.reshape(HPC, D, NKT, 128)
        khost = np.zeros((HPC, 128, NKT, 128), np.float16)
        khost[:, 0:D, 0::2] = kr[:, :, 0::2]
        khost[:, 64 : 64 + D, 1::2] = kr[:, :, 1::2]
        vh = (
            value[:, sl, :]
            .transpose(1, 0, 2)
            .reshape(HPC, NKT, 128, D)
            .astype(np.float16)
        )
        vhost = np.zeros((HPC, 128, NKT, 128), np.float16)
        vhost[:, :, :, 0:D] = vh.transpose(0, 2, 1, 3)
        vhost[:, :, :, D] = np.float16(1.0)
        in_maps.append(
            {
                "q": qhost,
                "k": khost.reshape(HPC, 128, S),
                "v": vhost,
            }
        )
    return in_maps


def kernel(query, key, value):
    global _NC_CACHE
    if _NC_CACHE is None:
        _NC_CACHE = _build()
    nc = _NC_CACHE

    in_maps = prepare_in_maps(query, key, value)
    res = run_bass_kernel_spmd(nc, in_maps, core_ids=list(range(NCORES)))
    out = np.concatenate(
        [res.results[c]["o"].transpose(1, 0, 2) for c in range(NCORES)], axis=1
    )
    return out


# revision 50
# speedup vs baseline: 1.0174x; 1.0048x over previous
"""Dense multi-head attention (S=4096, H=16, D=64) on 8 Trainium2 NeuronCores.

Sharding: heads split across cores (2 heads per core), no cross-core comms.

Host side: Q and K are pre-transposed per head to [D, S] (d-major) so the
kernel DMA-loads Q^T/K^T directly with 16KB-contiguous runs; V stays [S, D].
Q and K are additionally pre-scaled by G = sqrt(1024*log2(e)/sqrt(D)) so the
PSUM logits arrive as x = 1024*log2(e)*s' (s' = scaled logit), which both
exp paths consume directly.

Per-core kernel (per head):
  - QK runs in 64x128 row-tiled PE mode: even k-tiles' K^T stationaries live
    on SBUF/array rows 0-63, odd k-tiles' on rows 64-127, and Q^T is
    duplicated into both partition halves. The two 512-wide matmuls of a
    k-tile pair execute concurrently on the two array halves (contraction is
    only D=64), halving QK's PE time vs the padded-to-128 formulation.
  - exp of each pair's [128, 1024] PSUM group alternates between ScalarE
    (table exp, scale/bias fused) and a custom 8-block VectorE op
    (EXP2_FAST_ANT: magic-number round + quadratic exp2 mantissa polynomial,
    result written as int16 bit pattern = fp16 value). Both paths produce
    kappa*exp(s'): the common factor cancels in the softmax divide.
    Splitting exp across two engines removes the ScalarE bottleneck
    (baseline: ACT 256us busy of 304us total).
  - PV also runs row-tiled 64x128: each k-tile's 128-key contraction splits
    into keys 0-63 (array rows 0-63, accumulating into acc_lo) and keys
    64-127 (rows 64-127, acc_hi); the two matmuls execute concurrently, so
    the pair costs one 512-column stream. Keeping QK and PV in the same PE
    tiling mode avoids the ~250ns/pair LDWEIGHTS exposure that mode
    alternation costs (measured v3). V' has an appended ones-column so row
    64 of acc_lo+acc_hi is the softmax denominator. PV lags exp by two
    groups so the in-order PE stream never waits on an exp in flight.
  - Epilogue: acc_lo+acc_hi -> fp16 (ScalarE copy + VectorE add, one PSUM
    read each), XBAR DMA-transpose to [128 q, 4, 80], reciprocal of col 64,
    broadcast multiply, DMA out.

Pipeline notes (all measured on HW via neuron-profile traces):
  - The in-order PE queue forces PV to lag exp by two pair-groups, else PE
    idles on every exp's ~1.2us latency.
  - Input loads are ordered by consumption deadline (K chunks + V first).
    Host-side fp16 layout prep means zero on-device staging work: no casts,
    no memsets, no stg ring -- which also keeps the strict-FIFO sequencers
    free of head-of-line blocking hazards.
"""

import numpy as np

import concourse.mybir as mybir
import concourse.tile as tile
from concourse import bacc
from concourse.bass_utils import run_bass_kernel_spmd

S = 4096
H = 16
D = 64
NCORES = 8
HPC = H // NCORES  # heads per core
NKT = S // 128  # 32 k-tiles per head
NPAIR = NKT // 2  # 16 row-tiled k-tile pairs per head
NQC = S // 512  # 8 q chunks per head
NCH = NKT // 8  # 4 load chunks per head (1024 columns each)
SCALE = 1.0 / np.sqrt(D)

F32 = mybir.dt.float32
F16 = mybir.dt.float16
I16 = mybir.dt.int16

# ---- exp formulation constants -------------------------------------------
LOG2E = float(np.log2(np.e))
PRESCALE = float(np.sqrt(1024.0 * LOG2E * SCALE))  # host-applied to q and k
MAGIC = 12884901888.0  # 1.5 * 2^33: fp32 add rounds x to a multiple of 1024
EXP_A = 0.98531599  # quadratic exp2 mantissa poly (minimax incl. borrow)
EXP_B = -3.0719600e-4
MEAN_ADJ = 0.00577  # DVE op's mean rel error, matched in the ACT path
SCALE_ACT = float(np.log(2.0) / 1024.0)
# Both exp paths produce kappa*exp(s') with kappa = 1.01587 (the DVE fit's
# common scale); it cancels in the softmax divide.
BIAS_ACT = float(np.log(0.71833097 * np.sqrt(2.0)) + MEAN_ADJ)

# ---- custom DVE op: EXP2_FAST_ANT ----------------------------------------
# out_i16 = tb + P where (all fp32, one 8-block DVE pass, 1 elem/cycle/lane):
#   w  = x + MAGIC          # rounds x to nearest multiple of 1024
#   u  = w - MAGIC          # 1024*k
#   f  = x - u              # mantissa residual in [-512, 512)
#   tb = w - (MAGIC-15360)  # 1024*(15+k): fp16 exponent field
#   P  = f*(EXP_A + f*EXP_B)
# int16(out) bitcast fp16 == kappa * exp(s') to ~1.6% (sawtooth), which the
# softmax normalization reduces to ~6e-3 of output max.


def _exp_reference(in0, in1, s0, s1, imm2):
    x = in0.astype(np.float32)
    w = (x + np.float32(s0)).astype(np.float32)
    u = (w - np.float32(s0)).astype(np.float32)
    f = (x - u).astype(np.float32)
    tb = (w - np.float32(s1)).astype(np.float32)
    return (tb + f * (np.float32(imm2) + f * in1)).astype(np.float32)


def _register_exp_op():
    import concourse.dve_ops as dops
    from concourse.dve_spec import (
        C0,
        C1,
        C2,
        C3,
        Spec,
        Src0,
        _has_src1,
        _spill_c3_to_src1,
        lower,
    )
    from concourse.dve_uop import DveOpSpec

    name = "EXP2_FAST_ANT"
    for op in dops.OPS:
        if op.name == name:
            return op
    w = Src0 + C0
    u = w - C0
    f = Src0 - u
    tb = w - C1
    P = (f * C3 + C2) * f
    body = _spill_c3_to_src1(tb + P)
    spec = Spec(body=body, reference=_exp_reference)
    row = max(dops._SUB_OPCODE_FOR_NAME.values()) + 1
    assert row < 0x20
    dops._SUB_OPCODE_FOR_NAME[name] = row
    shas = {}
    for ver in ("v3", "v4"):
        tmp = DveOpSpec(
            name=name, opcode=row, uops=lower(spec, ver=ver), rd1_en=_has_src1(spec)
        )
        shas[ver] = tmp.sha(ver)
    op = dops.DveOp(name, spec, subdim=False, uops_sha=shas)
    dops.OPS.append(op)
    dops.CUSTOM_DVE_SPECS[name] = spec
    return op


EXP_OP = _register_exp_op()


EXP_PAT = "ADADADADADADADA"  # 8 ScalarE / 7 VectorE per 15 exp groups


def _build_head(nc, tc, pools, bco, bact, q, k, v, o, h):
    sb, epool, spsum, opsum = pools

    # ---- Phase A: direct fp16 DMAs; host delivers device-ready layouts ----
    # kt: even k-tiles' K^T on partitions 0..63, odd k-tiles' on 64..127
    # (host-interleaved). qt: Q^T duplicated into both partition halves
    # (host-duplicated). vstage: V' with ones column 64 and zero padding
    # (host-padded). Loads ordered by consumption deadline: every kts chunk
    # is read within q-chunk 0, V' feeds PV from slot 2, qts[b] waits until
    # q-chunk 2b.
    qts = [
        sb.tile([128, 1024], F16, tag=f"qt{b}", name=f"qt{b}") for b in range(NCH)
    ]
    kts = [
        sb.tile([128, 1024], F16, tag=f"kt{b}", name=f"kt{b}") for b in range(NCH)
    ]

    def qk_chunk(src_t, dsts, b, c0=0, w=1024):
        nc.sync.dma_start(
            dsts[b][:, c0 : c0 + w],
            src_t.ap()[h, :, b * 1024 + c0 : b * 1024 + c0 + w],
        )

    qk_chunk(k, kts, 0, c0=0, w=512)
    qk_chunk(q, qts, 0, c0=0, w=512)

    vstage = sb.tile([128, NKT, 128], F16, tag="vstage")
    nc.sync.dma_start(vstage[:], v.ap()[h])

    qk_chunk(k, kts, 0, c0=512, w=512)
    qk_chunk(k, kts, 1)
    qk_chunk(k, kts, 2)
    qk_chunk(k, kts, 3)
    qk_chunk(q, qts, 0, c0=512, w=512)
    for b in range(1, NCH):
        qk_chunk(q, qts, b)

    # ---- Phase B: attention, PV lagging exp by two pair-groups ----
    def qk_group(qc, p):
        t0 = 2 * p
        b = t0 // 8
        qb = qc // 2
        qcols = (qc * 512) % 1024
        sp = spsum.tile([128, 1024], F32, tag="sp")
        nc.tensor.matmul(
            sp[:, 0:512],
            kts[b][0:D, (t0 % 8) * 128 : (t0 % 8 + 1) * 128],
            qts[qb][0:D, qcols : qcols + 512],
        )
        nc.tensor.matmul(
            sp[:, 512:1024],
            kts[b][64 : 64 + D, (t0 % 8 + 1) * 128 : (t0 % 8 + 2) * 128],
            qts[qb][64 : 64 + D, qcols : qcols + 512],
        )
        return sp

    def epilogue(ot, qs):
        # XBAR DMA transpose: ot [80, 512] fp16 -> otT [128, 4, 80] where
        # query q = n*128 + p sits at otT[p, n, :] (rows 65..79 are junk pad).
        otT = sb.tile([128, 4, 80], F16, tag="otT", bufs=3)
        nc.sync.dma_start_transpose(otT[:], ot[:])
        fin = sb.tile([128, 4, D], F32, tag="fin", bufs=3)
        rcp = sb.tile([128, 4, 1], F32, tag="rcp")
        nc.vector.reciprocal(rcp[:, :, 0], otT[:, :, D])
        nc.vector.tensor_tensor(
            fin[:],
            otT[:, :, 0:D],
            rcp[:].to_broadcast([128, 4, D]),
            mybir.AluOpType.mult,
        )
        nc.sync.dma_start(
            o.ap()[h, qs : qs + 512, :].rearrange("(n p) d -> p n d", p=128),
            fin[:],
        )

    groups = [(qc, p) for qc in range(NQC) for p in range(NPAIR)]
    n = len(groups)
    ets = {}
    state = {"acc": None, "pending": None}

    def issue_exp(i, sp):
        qc, p = groups[i]
        et = epool.tile([128, 1024], F16, tag="et")
        if EXP_PAT[i % len(EXP_PAT)] == "D":
            # VectorE path: custom 8-block exp, int16 bit pattern = fp16
            nc.vector._custom_dve(
                EXP_OP,
                out=et[:].bitcast(I16),
                in0=sp[:],
                in1=bco[:, 0:1],
                s0=MAGIC,
                s1=MAGIC - 15360.0,
                imm2=EXP_A,
            )
        else:
            nc.scalar.activation(
                et[:],
                sp[:],
                mybir.ActivationFunctionType.Exp,
                scale=SCALE_ACT,
                bias=bact[:, 0:1],
            )
        ets[i] = et

    def issue_pv(i):
        qc, p = groups[i]
        et = ets.pop(i)
        if p == 0:
            if state["pending"] is not None:
                epilogue(*state["pending"])
                state["pending"] = None
            state["acc"] = (
                opsum.tile([128, 512], F32, tag="acc", name="acc_lo"),
                opsum.tile([128, 512], F32, tag="acc", name="acc_hi"),
            )
        acc_lo, acc_hi = state["acc"]
        for j in range(2):
            t = 2 * p + j
            # Row-tiled PV pair: keys 0-63 on array rows 0-63 -> acc_lo,
            # keys 64-127 on rows 64-127 -> acc_hi; runs concurrently.
            nc.tensor.matmul(
                acc_lo[:],
                vstage[0:64, t, :],
                et[0:64, j * 512 : (j + 1) * 512],
                start=(t == 0),
                stop=(t == NKT - 1),
            )
            nc.tensor.matmul(
                acc_hi[:],
                vstage[64:128, t, :],
                et[64:128, j * 512 : (j + 1) * 512],
                start=(t == 0),
                stop=(t == NKT - 1),
            )
        if p == NPAIR - 1:
            # merge the two half-contraction accumulators; frees both banks
            # (an instruction may read only ONE input from PSUM: ScalarE
            # copies acc_lo out, VectorE adds acc_hi)
            ot = sb.tile([80, 512], F16, tag="ot", bufs=3)
            nc.scalar.copy(ot[0 : D + 1, :], acc_lo[0 : D + 1, :])
            nc.vector.tensor_tensor(
                ot[0 : D + 1, :],
                ot[0 : D + 1, :],
                acc_hi[0 : D + 1, :],
                mybir.AluOpType.add,
            )
            state["pending"] = (ot, qc * 512)

    # PV lags exp by two groups so the in-order PE stream (QK(i+1), PV(i-2))
    # never waits on an exp still in flight on ScalarE/VectorE.
    sp_next = qk_group(*groups[0])
    pvq = []
    for i in range(n):
        issue_exp(i, sp_next)
        if i + 1 < n:
            sp_next = qk_group(*groups[i + 1])
        if i >= 2:
            pvq.append(i - 2)
            # Hold a chunk-first PV one extra slot: one more QK pair runs
            # before the PE needs the acc banks the merge chain is freeing.
            while pvq and not (
                len(pvq) == 1 and groups[pvq[0]][1] == 0 and i + 1 < n
            ):
                issue_pv(pvq.pop(0))
    for j in pvq + [n - 2, n - 1]:
        issue_pv(j)
    epilogue(*state["pending"])


def _build():
    nc = bacc.Bacc(trn_type="TRN2", debug=False, num_devices=NCORES)
    q = nc.dram_tensor("q", [HPC, 128, S], F16, kind="ExternalInput")
    k = nc.dram_tensor("k", [HPC, 128, S], F16, kind="ExternalInput")
    v = nc.dram_tensor("v", [HPC, 128, NKT, 128], F16, kind="ExternalInput")
    o = nc.dram_tensor("o", [HPC, S, D], F32, kind="ExternalOutput")

    with tile.TileContext(nc) as tc:
        with (
            tc.tile_pool(name="const", bufs=1) as cpool,
            tc.tile_pool(name="sb", bufs=2) as sb,
            tc.tile_pool(name="epool", bufs=6) as epool,
            tc.tile_pool(name="spsum", bufs=3, space="PSUM") as spsum,
            tc.tile_pool(name="opsum", bufs=2, space="PSUM") as opsum,
        ):
            # Dummy exp at t~0 pulls the ACT table-load DMA in front of the
            # input DMAs (otherwise the first input chunk queues behind it).
            warm = cpool.tile([128, 1], F32, tag="warm")
            nc.gpsimd.memset(warm[:], 0.0)
            nc.scalar.activation(
                warm[:], warm[:], mybir.ActivationFunctionType.Exp
            )
            bco = cpool.tile([128, 1], F32, tag="bco")
            nc.gpsimd.memset(bco[:], EXP_B)
            bact = cpool.tile([128, 1], F32, tag="bact")
            nc.gpsimd.memset(bact[:], BIAS_ACT)
            pools = (sb, epool, spsum, opsum)
            for h in range(HPC):
                _build_head(nc, tc, pools, bco, bact, q, k, v, o, h)

    nc.compile()
    return nc


_NC_CACHE = None


def prepare_in_maps(query, key, value):
    """Host prep: per-core slices, prescale q/k, cast fp16, and lay out in
    the exact SBUF formats (K even/odd row-interleave, Q duplicated halves,
    V' ones-column padded) so the kernel loads with plain DMAs."""
    query = np.asarray(query)
    key = np.asarray(key)
    value = np.asarray(value)
    g = np.float32(PRESCALE)
    in_maps = []
    for c in range(NCORES):
        sl = slice(c * HPC, (c + 1) * HPC)
        qh = ((query[:, sl, :] * g).transpose(1, 2, 0)).astype(np.float16)
        kh = ((key[:, sl, :] * g).transpose(1, 2, 0)).astype(np.float16)
        qhost = np.ascontiguousarray(np.concatenate([qh, qh], axis=1))
        kr = kh.reshape(HPC, D, NKT, 128)
        khost = np.zeros((HPC, 128, NKT, 128), np.float16)
        khost[:, 0:D, 0::2] = kr[:, :, 0::2]
        khost[:, 64 : 64 + D, 1::2] = kr[:, :, 1::2]
        vh = (
            value[:, sl, :]
            .transpose(1, 0, 2)
            .reshape(HPC, NKT, 128, D)
            .astype(np.float16)
        )
        vhost = np.zeros((HPC, 128, NKT, 128), np.float16)
        vhost[:, :, :, 0:D] = vh.transpose(0, 2, 1, 3)
        vhost[:, :, :, D] = np.float16(1.0)
        in_maps.append(
            {
                "q": qhost,
                "k": khost.reshape(HPC, 128, S),
                "v": vhost,
            }
        )
    return in_maps


def kernel(query, key, value):
    global _NC_CACHE
    if _NC_CACHE is None:
        _NC_CACHE = _build()
    nc = _NC_CACHE

    in_maps = prepare_in_maps(query, key, value)
    res = run_bass_kernel_spmd(nc, in_maps, core_ids=list(range(NCORES)))
    out = np.concatenate(
        [res.results[c]["o"].transpose(1, 0, 2) for c in range(NCORES)], axis=1
    )
    return out
